# revision 4
# baseline (speedup 1.0000x reference)
"""Composite loss (boundary-weighted BCE + Dice) Trainium2 kernel.

Full inputs: pred (32,1,512,512) f32, target (32,1,512,512) i32.
Data-parallel over 8 NeuronCores (4 images per core). Each core computes
three partial sums; the host combines them into (total, bce, dice).

Per-core math (B_loc=4 images, each 512x512, t binary):
  x   = pred + t                       -> sum(x) = sum(pred) + sum(t)  [dice denom]
  pt  = Relu(x - 1) = pred * t         -> sum(pt) = intersection
  q   = max(|x - 1|, 1e-7)             -> = clip(t ? p : 1-p, eps, ~)
  L   = ln(q)                          (bce_map = -L)
  s9  = 3x3 clamp-padded window sum of t   (TensorE band matmuls)
  w   = 3 - 2*relu(|s9 - 4.5| - 3.5)   (= 3 on boundary pixels, else 1)
  swL = sum(w * L)                     -> bce = -swL / N
"""

import sys

sys.path.insert(0, "/opt/trn_rl_repo")

from contextlib import ExitStack

import numpy as np

N_CORES = 8
B, H, W = 32, 512, 512
B_LOC = B // N_CORES          # 4 images per core
P = 128                       # partitions
NBLK = H // P                 # 4 row-blocks per image
IMG_F = NBLK * W              # 2048 free-dim elements per image tile
N_TOTAL = float(B * H * W)
EPS = 1e-7
SMOOTH = 1e-6

_PROGRAM = None


def _build_consts():
    import ml_dtypes

    # Vertical tridiagonal band matrices (lhsT layout: [k_in, m_out]).
    band_mid = np.zeros((P, P), dtype=np.float32)
    for k in range(P):
        for m in range(max(0, k - 1), min(P, k + 2)):
            band_mid[k, m] = 1.0
    band_top = band_mid.copy()
    band_top[0, 0] += 1.0      # clamp-replicate image row 0
    band_bot = band_mid.copy()
    band_bot[P - 1, P - 1] += 1.0  # clamp-replicate image row 511
    # Per-block halo selector lhsT (K=6 halo rows, M=128 out rows).
    # Halo row layout per image: [b0r127, b1r0, b1r127, b2r0, b2r127, b3r0].
    # Block b's out row 0 takes halo row 2(b-1) (= row above), out row 127
    # takes halo row 2b+1 (= row below).
    nblk = 4
    hsel = np.zeros((nblk, 2 * (nblk - 1), P), dtype=np.float32)
    for b in range(nblk):
        if b > 0:
            hsel[b, 2 * (b - 1), 0] = 1.0
        if b < nblk - 1:
            hsel[b, 2 * b + 1, P - 1] = 1.0
    bf = ml_dtypes.bfloat16
    return {
        "band_top": band_top.astype(bf),
        "band_mid": band_mid.astype(bf),
        "band_bot": band_bot.astype(bf),
        "hsel": np.ascontiguousarray(
            hsel.reshape(nblk * 2 * (nblk - 1), P)).astype(bf),
    }


def _build_program():
    import concourse.bacc as bacc
    import concourse.tile as tile
    from concourse import mybir

    AF = mybir.ActivationFunctionType
    ALU = mybir.AluOpType
    dt = mybir.dt

    nc = bacc.Bacc("TRN2", target_bir_lowering=False, debug=False,
                   num_devices=N_CORES)

    pred_d = nc.dram_tensor("pred", (B_LOC * H, W), dt.float32,
                            kind="ExternalInput").ap()
    tgt_d = nc.dram_tensor("tgt", (B_LOC * H, W), dt.int32,
                           kind="ExternalInput").ap()
    band_top_d = nc.dram_tensor("band_top", (P, P), dt.bfloat16,
                                kind="ExternalInput").ap()
    band_mid_d = nc.dram_tensor("band_mid", (P, P), dt.bfloat16,
                                kind="ExternalInput").ap()
    band_bot_d = nc.dram_tensor("band_bot", (P, P), dt.bfloat16,
                                kind="ExternalInput").ap()
    hsel_d = nc.dram_tensor("hsel", (NBLK * 2 * (NBLK - 1), P), dt.bfloat16,
                            kind="ExternalInput").ap()

    o_accx = nc.dram_tensor("o_accx", (P, B_LOC), dt.float32,
                            kind="ExternalOutput").ap()
    o_accq0 = nc.dram_tensor("o_accq0", (P, B_LOC), dt.float32,
                             kind="ExternalOutput").ap()
    o_accl = nc.dram_tensor("o_accl", (P, B_LOC), dt.float32,
                            kind="ExternalOutput").ap()
    o_accwl = nc.dram_tensor("o_accwl", (P, B_LOC), dt.float32,
                             kind="ExternalOutput").ap()

    # const APs for activation bias values
    def register_const_ap(dtype, value):
        t = nc.alloc_sbuf_tensor(f"const-{dtype.name}-{value}", [128, 1], dtype)
        nc.gpsimd.memset(t.ap(), value)
        nc.const_aps.aps[(dtype, value)] = t.ap()

    for v in (-1.0, -4.5):
        register_const_ap(dt.float32, v)
    nc.all_engine_barrier()

    with tile.TileContext(nc) as tc:
        with ExitStack() as ctx:
            cpool = ctx.enter_context(tc.tile_pool(name="consts", bufs=1))
            inpool = ctx.enter_context(tc.tile_pool(name="inp", bufs=2))
            mid = ctx.enter_context(tc.tile_pool(name="mid", bufs=2))
            accp = ctx.enter_context(tc.tile_pool(name="acc", bufs=1))
            psum = ctx.enter_context(
                tc.tile_pool(name="psum", bufs=2, space="PSUM"))

            # constants
            band_t = cpool.tile([P, P], dt.bfloat16, tag="btop")
            nc.sync.dma_start(band_t[:], band_top_d[:])
            band_m = cpool.tile([P, P], dt.bfloat16, tag="bmid")
            nc.sync.dma_start(band_m[:], band_mid_d[:])
            band_b = cpool.tile([P, P], dt.bfloat16, tag="bbot")
            nc.sync.dma_start(band_b[:], band_bot_d[:])
            # one (6, 128) selector tile per block, each based at partition 0
            hsel_ts = []
            for b in range(NBLK):
                hse = cpool.tile([2 * (NBLK - 1), P], dt.bfloat16,
                                 tag=f"hsel{b}")
                nc.sync.dma_start(
                    hse[:], hsel_d[b * 2 * (NBLK - 1):(b + 1) * 2 * (NBLK - 1), :])
                hsel_ts.append(hse)
            bands = [band_t, band_m, band_m, band_b]

            # per-core accumulators (one column per image)
            accx = accp.tile([P, B_LOC], dt.float32, tag="accx")
            accq0 = accp.tile([P, B_LOC], dt.float32, tag="accq0")
            accl = accp.tile([P, B_LOC], dt.float32, tag="accl")
            accwl = accp.tile([P, B_LOC], dt.float32, tag="accwl")

            for g in range(B_LOC):
                rows = slice(g * H, (g + 1) * H)

                p_img = inpool.tile([P, IMG_F], dt.float32, tag="p")
                nc.sync.dma_start(
                    p_img[:].rearrange("p (n m) -> p n m", m=W),
                    pred_d[rows, :].rearrange("(n p) m -> p n m", p=P),
                )
                t_img = inpool.tile([P, IMG_F], dt.int32, tag="t")
                nc.sync.dma_start(
                    t_img[:].rearrange("p (n m) -> p n m", m=W),
                    tgt_d[rows, :].rearrange("(n p) m -> p n m", p=P),
                )

                # halo rows (image-local rows 127,128 | 255,256 | 383,384),
                # pairs are contiguous in DRAM
                h_i32 = mid.tile([2 * (NBLK - 1), W], dt.int32, tag="hraw")
                for b in range(NBLK - 1):
                    r0 = g * H + (b + 1) * P - 1
                    nc.sync.dma_start(h_i32[2 * b:2 * b + 2, :],
                                      tgt_d[r0:r0 + 2, :])

                # int32 -> bf16 conversions (GPSIMD)
                tb = mid.tile([P, IMG_F], dt.bfloat16, tag="tb")
                nc.gpsimd.tensor_copy(tb[:], t_img[:])
                hb = mid.tile([2 * (NBLK - 1), W], dt.bfloat16, tag="hb")
                nc.gpsimd.tensor_copy(hb[:], h_i32[:])

                # horizontal 3-window clamp sum of halo rows (GPSIMD)
                nh = 2 * (NBLK - 1)
                ha = mid.tile([nh, W], dt.bfloat16, tag="ha")
                hs = mid.tile([nh, W], dt.bfloat16, tag="hs")
                # a[n] = h[n] + h[n+1], n in [0, W-2]
                nc.gpsimd.tensor_add(ha[:, 0:W - 1], hb[:, 0:W - 1],
                                     hb[:, 1:W])
                # hs[n] = a[n-1] + h[n+1], n in [1, W-2]
                nc.gpsimd.tensor_add(hs[:, 1:W - 1], ha[:, 0:W - 2],
                                     hb[:, 2:W])
                # hs[0] = a[0] + h[0];  hs[W-1] = a[W-2] + h[W-1]
                nc.gpsimd.tensor_add(hs[:, 0:1], ha[:, 0:1], hb[:, 0:1])
                nc.gpsimd.tensor_add(hs[:, W - 1:W], ha[:, W - 2:W - 1],
                                     hb[:, W - 1:W])

                # x = pred + t, accumulate sum(x)
                x = mid.tile([P, IMG_F], dt.float32, tag="x")
                nc.vector.scalar_tensor_tensor(
                    out=x[:], in0=p_img[:], scalar=0.0, in1=tb[:],
                    op0=ALU.bypass, op1=ALU.add,
                    accum_out=accx[:, g:g + 1],
                )

                # q = max(|x-1|, eps); L = ln(q).
                # sum(|x-1|) is accumulated for free; the host derives the
                # intersection: sum(relu(x-1)) = (sum(x) - N + sum|x-1|)/2.
                q = mid.tile([P, IMG_F], dt.float32, tag="q")
                nc.scalar.activation(q[:], x[:], AF.Abs, bias=-1.0, scale=1.0,
                                     accum_out=accq0[:, g:g + 1])
                nc.vector.tensor_scalar_max(q[:], q[:], EPS)
                L = mid.tile([P, IMG_F], dt.float32, tag="L")
                nc.scalar.activation(L[:], q[:], AF.Ln,
                                     accum_out=accl[:, g:g + 1])

                # s9: 3x3 clamp-padded window sum via band matmuls
                s9 = psum.tile([P, IMG_F], dt.float32, tag="s9")
                for b in range(NBLK):
                    cs = b * W
                    blk = slice(cs, cs + W)
                    tbb = tb[:, blk]
                    bd = bands[b]
                    nc.tensor.matmul(s9[:, blk], bd[:], tbb[:],
                                     start=True, stop=False)
                    nc.tensor.matmul(s9[:, cs + 1:cs + W], bd[:],
                                     tbb[:, 0:W - 1], start=False, stop=False)
                    nc.tensor.matmul(s9[:, cs:cs + W - 1], bd[:],
                                     tbb[:, 1:W], start=False, stop=False)
                    # horizontal clamp corrections (cols 0 and W-1)
                    nc.tensor.matmul(s9[:, cs:cs + 1], bd[:], tbb[:, 0:1],
                                     start=False, stop=False)
                    nc.tensor.matmul(s9[:, cs + W - 1:cs + W], bd[:],
                                     tbb[:, W - 1:W], start=False, stop=False)
                    # vertical halo rows from neighboring blocks (K=6 select)
                    nc.tensor.matmul(s9[:, blk], hsel_ts[b][:], hs[:],
                                     start=False, stop=True)

                # notb = relu(|s9-4.5| - 3.5): 1 on uniform windows, else 0.
                # Host combines: sum(w*L) = 3*sum(L) - 2*sum(notb*L).
                u = mid.tile([P, IMG_F], dt.bfloat16, tag="u")
                nc.scalar.activation(u[:], s9[:], AF.Abs, bias=-4.5, scale=1.0)
                nb = mid.tile([P, IMG_F], dt.bfloat16, tag="nb")
                nc.vector.tensor_scalar(
                    out=nb[:], in0=u[:], scalar1=3.5, scalar2=0.0,
                    op0=ALU.subtract, op1=ALU.max)

                # sum(notb * L)
                junk2 = mid.tile([P, IMG_F], dt.float32, tag="junk2")
                nc.vector.scalar_tensor_tensor(
                    out=junk2[:], in0=L[:], scalar=0.0, in1=nb[:],
                    op0=ALU.bypass, op1=ALU.mult,
                    accum_out=accwl[:, g:g + 1],
                )

            nc.sync.dma_start(o_accx[:], accx[:])
            nc.sync.dma_start(o_accq0[:], accq0[:])
            nc.sync.dma_start(o_accl[:], accl[:])
            nc.sync.dma_start(o_accwl[:], accwl[:])

    nc.compile()
    return nc


def _get_program():
    global _PROGRAM
    if _PROGRAM is None:
        _PROGRAM = _build_program()
    return _PROGRAM


def kernel(pred, target, _want_results=False, _trace=False):
    from concourse.bass_utils import run_bass_kernel_spmd

    pred = np.asarray(pred, dtype=np.float32).reshape(B, H, W)
    target = np.asarray(target, dtype=np.int32).reshape(B, H, W)

    consts = _build_consts()
    nc = _get_program()

    in_maps = []
    for c in range(N_CORES):
        sl = slice(c * B_LOC, (c + 1) * B_LOC)
        in_maps.append({
            "pred": np.ascontiguousarray(
                pred[sl].reshape(B_LOC * H, W)),
            "tgt": np.ascontiguousarray(
                target[sl].reshape(B_LOC * H, W)),
            **consts,
        })

    res = run_bass_kernel_spmd(nc, in_maps, list(range(N_CORES)),
                               trace=_trace)

    sx = 0.0
    sq0 = 0.0
    sl = 0.0
    snl = 0.0
    for c in range(N_CORES):
        r = res.results[c]
        sx += float(np.asarray(r["o_accx"], np.float64).sum())
        sq0 += float(np.asarray(r["o_accq0"], np.float64).sum())
        sl += float(np.asarray(r["o_accl"], np.float64).sum())
        snl += float(np.asarray(r["o_accwl"], np.float64).sum())

    # relu(v) = (v + |v|)/2  =>  sum(pred*t) = (sum(x) - N + sum|x-1|)/2
    spt = (sx - N_TOTAL + sq0) / 2.0
    # w = 3 - 2*notb  =>  sum(w*L) = 3*sum(L) - 2*sum(notb*L)
    swl = 3.0 * sl - 2.0 * snl

    bce = -swl / N_TOTAL
    dice = 1.0 - (2.0 * spt + SMOOTH) / (sx + SMOOTH)
    total = 0.5 * bce + 0.5 * dice

    out = (np.float32(total), np.float32(bce), np.float32(dice))
    if _want_results:
        return out, res
    return out
